# revision 20
# baseline (speedup 1.0000x reference)
"""Kernel ridge regression on 8 TRN2 NeuronCores.

Math:
  K = exp(-g*||xi-xj||^2), A = K + I, dual = A^{-1} y, out = K@dual = y - dual.
  Diagonal similarity: A = D (E + D^{-2}) D with D = diag(exp(-g*|xi|^2)),
  E = exp(2g * X X^T).  Solve B v = D^{-1} y by batched CG (B = E + D^{-2}),
  then dual = D^{-1} v, and D^{-1} = exp(+g*|xi|^2).
Sharding: rows split 8 ways (512 rows/core). Each core holds the E block
  [4096(j, contraction), 512(i, its rows)] in SBUF as 32 chunks [128, 512].
  Matvec: lhsT = p chunk [128,32] (weights), rhs = E chunk (free 512)
  -> psum [32, 512] = (E p)^T slice; PE-transpose back; diag added locally.
  Per iteration: AllGather(p slices) + 2 tiny AllReduce (dots).

Host<->device traffic is the wall-clock bottleneck through the axon tunnel
(~70 ms round-trip latency + ~100 MB/s bandwidth); device exec is a few ms.
So per call the host uploads only the row-sharded bf16 inputs X [512,256]
and y [512,32] per core (~2.25 MB total). Each core PE-transposes its own
X shard and the cores AllGather X^T on-device (instead of uploading the
full X^T replicated 8x = 32 MB). The 128x128 identity used by PE
transposes is synthesized with affine_select. The output is AllGather'd
on-device so every core holds the full [4096, 32] result (bf16) and the
host fetches a single shard. The executable is AOT-compiled once with
fast_dispatch_compile (C++ fast-path dispatch, no per-call retrace), and
the donated output scratch reuses the previous call's device-resident
output (fully overwritten each call, so no zero upload on warm calls).
"""

import sys

sys.path.insert(0, "/opt/trn_rl_repo")

import numpy as np

import concourse.bacc as bacc
import concourse.bass as bass
import concourse.mybir as mybir
import concourse.tile as tile
from concourse.masks import make_identity

N, D, T = 4096, 256, 32
C = 8
R = N // C  # 512 rows per core
GAMMA = 1.0 / 256.0
NITER = 16

F32 = mybir.dt.float32
BF16 = mybir.dt.bfloat16
Exp = mybir.ActivationFunctionType.Exp
ADD = mybir.AluOpType.add
MULT = mybir.AluOpType.mult
BYPASS = mybir.AluOpType.bypass
RG = [list(range(C))]

_CACHE = {}


def _build(niter):
    nc = bacc.Bacc("TRN2", target_bir_lowering=False, debug=False, num_devices=C)
    xb_d = nc.dram_tensor("xb", [R, D], BF16, kind="ExternalInput").ap()
    y_d = nc.dram_tensor("yv", [R, T], BF16, kind="ExternalInput").ap()
    out_d = nc.dram_tensor("out", [N, T], BF16, kind="ExternalOutput").ap()

    with tile.TileContext(nc) as tc:
        _body(tc, niter, xb_d, y_d, out_d)
    nc.compile()
    return nc


def _body(tc, niter, xb_d, y_d, out_d):
    nc = tc.nc
    with (
        tc.tile_pool(name="big", bufs=1) as big,
        tc.tile_pool(name="work", bufs=4) as work,
        tc.tile_pool(name="pp", bufs=1, space="PSUM") as pp,
        tc.tile_pool(name="dram", bufs=1, space="DRAM") as dp,
    ):
        # ---------------- persistent SBUF ----------------
        XT = big.tile([128, 2 * N], F32)  # X^T, d-half h at cols h*N
        XTC = big.tile([128, 2 * R], F32)  # local X^T block (d-half h at h*R)
        E = big.tile([128, 32 * 512], F32)  # E row-block, j-chunk jc at jc*512
        xcs = big.tile([128, 4 * D], F32)  # local X rows (4 chunks), f32
        ys = big.tile([128, 4 * T], F32)  # local y rows (4 chunks)
        x2 = big.tile([128, 4], F32)
        esc = big.tile([128, 4], F32)  # exp(+g x2) local
        dg = big.tile([128, 4], F32)  # exp(2g x2) local (diag of B)
        xs = big.tile([128, 4 * T], F32)  # CG x
        rs = big.tile([128, 4 * T], F32)  # CG r
        ps = big.tile([128, 4 * T], F32)  # CG p (local slice)
        pf = big.tile([128, 32 * T], F32)  # p full (gathered), chunk jc at jc*T
        pf_raw = big.tile([128, 32 * T], F32)  # DMA landing zone for pf
        qs = big.tile([128, 4 * T], F32)  # q = B p local rows
        ones_c = big.tile([128, 1], F32)
        ones_r = big.tile([1, 128], F32)
        idn = big.tile([128, 128], F32)
        mu = big.tile([1, T], F32)
        sc = big.tile([1, 8 * T], F32)  # small scalar scratch

        def xc(k):  # local X rows, chunk k: [128, D]
            return xcs[:, k * D : (k + 1) * D]

        def yc(k):  # local y rows, chunk k: [128, T]
            return ys[:, k * T : (k + 1) * T]

        # ---------------- loads ----------------
        # Matmul (LDWEIGHTS) instructions tolerate very few semaphore waits, so
        # every matmul operand is staged through a DVE copy: DMA -> _raw tile
        # -> vector.tensor_copy -> tile consumed by the matmul. Consecutive DVE
        # ops collapse to a single wait for the consumer. The copy also widens
        # the bf16 X upload back to f32.
        xb_raw = big.tile([128, 4 * D], BF16)
        y_raw = big.tile([128, 4 * T], BF16)
        for k in range(4):
            nc.sync.dma_start(
                xb_raw[:, k * D : (k + 1) * D], xb_d[k * 128 : (k + 1) * 128, :]
            )
            nc.vector.tensor_copy(
                xcs[:, k * D : (k + 1) * D], xb_raw[:, k * D : (k + 1) * D]
            )
            nc.sync.dma_start(
                y_raw[:, k * T : (k + 1) * T], y_d[k * 128 : (k + 1) * 128, :]
            )
            nc.vector.tensor_copy(
                ys[:, k * T : (k + 1) * T], y_raw[:, k * T : (k + 1) * T]
            )
        make_identity(nc, idn[:])
        nc.vector.memset(ones_c[:], 1.0)
        nc.vector.memset(ones_r[:], 1.0)
        nc.vector.memset(xs[:], 0.0)

        # ---------------- local X^T via PE transpose ----------------
        for k in range(4):
            for h in range(2):
                tp = pp.tile([128, 512], F32, tag="mm", bufs=2)
                nc.tensor.transpose(
                    tp[:, 0:128], xc(k)[:, h * 128 : (h + 1) * 128], idn[:]
                )
                nc.vector.tensor_copy(
                    XTC[:, h * R + k * 128 : h * R + (k + 1) * 128], tp[:, 0:128]
                )

        # ---------------- AllGather X^T across cores ----------------
        agx_in = dp.tile([D, R], F32, name="agx_in")
        agx_out = dp.tile([C * D, R], F32, addr_space="Shared", name="agx_out")
        for h in range(2):
            nc.sync.dma_start(
                agx_in[h * 128 : (h + 1) * 128, :], XTC[:, h * R : (h + 1) * R]
            )
        nc.gpsimd.collective_compute(
            "AllGather",
            BYPASS,
            replica_groups=RG,
            ins=[agx_in.opt()],
            outs=[agx_out.opt()],
        )
        XT_raw = big.tile([128, 2 * N], F32)
        for c in range(C):
            for h in range(2):
                nc.sync.dma_start(
                    XT_raw[:, h * N + c * R : h * N + (c + 1) * R],
                    agx_out[c * D + h * 128 : c * D + (h + 1) * 128, :],
                )
                nc.vector.tensor_copy(
                    XT[:, h * N + c * R : h * N + (c + 1) * R],
                    XT_raw[:, h * N + c * R : h * N + (c + 1) * R],
                )

        # ---------------- x2 / scalings / init state ----------------
        for k in range(4):
            tmp = work.tile([128, D], F32, tag="xsq")
            nc.vector.tensor_mul(tmp[:], xc(k), xc(k))
            nc.vector.tensor_reduce(
                x2[:, k : k + 1], tmp[:], mybir.AxisListType.X, ADD
            )
        nc.scalar.activation(esc[:], x2[:], Exp, scale=GAMMA)
        nc.scalar.activation(dg[:], x2[:], Exp, scale=2 * GAMMA)
        for k in range(4):
            nc.vector.tensor_scalar(
                rs[:, k * T : (k + 1) * T], yc(k), esc[:, k : k + 1], None, MULT
            )
        nc.vector.tensor_copy(ps[:], rs[:])

        # ---------------- E construction ----------------
        for jc in range(32):
            g = pp.tile([128, 512], F32, tag="mm", bufs=2)
            nc.tensor.matmul(
                g[:],
                lhsT=XT[:, jc * 128 : (jc + 1) * 128],
                rhs=XTC[:, 0:R],
                start=True,
                stop=False,
            )
            nc.tensor.matmul(
                g[:],
                lhsT=XT[:, N + jc * 128 : N + (jc + 1) * 128],
                rhs=XTC[:, R : 2 * R],
                start=False,
                stop=True,
            )
            nc.scalar.activation(
                E[:, jc * 512 : (jc + 1) * 512], g[:], Exp, scale=2 * GAMMA
            )

        # ---------------- helpers ----------------
        def dot_partial(a, b, out_sb):
            """out_sb[1,T] = sum over local rows of a*b, per rhs column."""
            dps = pp.tile([1, T], F32, tag="dot", bufs=1)
            for k in range(4):
                m = work.tile([128, T], F32, tag="dm")
                nc.vector.tensor_mul(
                    m[:], a[:, k * T : (k + 1) * T], b[:, k * T : (k + 1) * T]
                )
                nc.tensor.matmul(
                    dps[:], lhsT=ones_c[:], rhs=m[:], start=(k == 0), stop=(k == 3)
                )
            nc.vector.tensor_copy(out_sb, dps[:])

        def allreduce(src_sb, dst_sb):
            ar_in = dp.tile([1, T], F32, name="ar_in")
            ar_out = dp.tile([1, T], F32, addr_space="Shared", name="ar_out")
            nc.sync.dma_start(ar_in[:], src_sb)
            nc.gpsimd.collective_compute(
                "AllReduce",
                ADD,
                replica_groups=RG,
                ins=[ar_in.opt()],
                outs=[ar_out.opt()],
            )
            nc.sync.dma_start(dst_sb, ar_out[:])

        def allgather_p():
            ag_in = dp.tile([R, T], F32, name="ag_in")
            ag_out = dp.tile([N, T], F32, addr_space="Shared", name="ag_out")
            nc.sync.dma_start(
                ag_in[:].rearrange("(k p) t -> p k t", p=128),
                ps[:].rearrange("p (k t) -> p k t", t=T),
            )
            nc.gpsimd.collective_compute(
                "AllGather",
                BYPASS,
                replica_groups=RG,
                ins=[ag_in.opt()],
                outs=[ag_out.opt()],
            )
            for k in range(4):
                nc.sync.dma_start(
                    pf_raw[:, k * 8 * T : (k + 1) * 8 * T].rearrange(
                        "p (c t) -> p c t", t=T
                    ),
                    ag_out[k * 1024 : (k + 1) * 1024, :].rearrange(
                        "(c p) t -> p c t", p=128
                    ),
                )
                nc.vector.tensor_copy(
                    pf[:, k * 8 * T : (k + 1) * 8 * T],
                    pf_raw[:, k * 8 * T : (k + 1) * 8 * T],
                )

        def bcast(vec_1xT, tag):
            b = pp.tile([128, T], F32, tag=tag, bufs=2)
            nc.tensor.matmul(b[:], lhsT=ones_r[:], rhs=vec_1xT, start=True, stop=True)
            return b

        # ---------------- CG init ----------------
        dot_partial(rs[:], rs[:], sc[:, 0:T])
        allreduce(sc[:, 0:T], mu[:])
        allgather_p()

        # ---------------- CG loop ----------------
        for it in range(niter):
            # q = E p (transposed slice), via 32 accumulating matmuls
            qt = pp.tile([32, 512], F32, tag="mm", bufs=2)
            for jc in range(32):
                nc.tensor.matmul(
                    qt[:],
                    lhsT=pf[:, jc * T : (jc + 1) * T],
                    rhs=E[:, jc * 512 : (jc + 1) * 512],
                    start=(jc == 0),
                    stop=(jc == 31),
                )
            qts = work.tile([32, 512], F32, tag="qts")
            nc.vector.tensor_copy(qts[:], qt[:])
            for k in range(4):
                tp = pp.tile([128, T], F32, tag="tp", bufs=2)
                nc.tensor.transpose(
                    tp[:], qts[:, k * 128 : (k + 1) * 128], idn[0:32, 0:32]
                )
                # q = diag*p + (E p)
                nc.vector.tensor_scalar(
                    qs[:, k * T : (k + 1) * T],
                    ps[:, k * T : (k + 1) * T],
                    dg[:, k : k + 1],
                    None,
                    MULT,
                )
                nc.vector.tensor_add(
                    qs[:, k * T : (k + 1) * T], qs[:, k * T : (k + 1) * T], tp[:]
                )
            # alpha = mu / (p.q)
            dot_partial(ps[:], qs[:], sc[:, T : 2 * T])
            allreduce(sc[:, T : 2 * T], sc[:, 2 * T : 3 * T])
            nc.vector.reciprocal(sc[:, 3 * T : 4 * T], sc[:, 2 * T : 3 * T])
            nc.vector.tensor_mul(sc[:, 4 * T : 5 * T], mu[:], sc[:, 3 * T : 4 * T])
            ab = bcast(sc[:, 4 * T : 5 * T], "bc")
            for k in range(4):
                s = slice(k * T, (k + 1) * T)
                t1 = work.tile([128, T], F32, tag="t1")
                nc.vector.tensor_mul(t1[:], ab[:], ps[:, s])
                nc.vector.tensor_add(xs[:, s], xs[:, s], t1[:])
                t2 = work.tile([128, T], F32, tag="t2")
                nc.vector.tensor_mul(t2[:], ab[:], qs[:, s])
                nc.vector.tensor_sub(rs[:, s], rs[:, s], t2[:])
            if it == niter - 1:
                break
            # beta = mu_new / mu
            dot_partial(rs[:], rs[:], sc[:, 5 * T : 6 * T])
            allreduce(sc[:, 5 * T : 6 * T], sc[:, 6 * T : 7 * T])
            nc.vector.reciprocal(sc[:, 7 * T : 8 * T], mu[:])
            nc.vector.tensor_mul(
                sc[:, 7 * T : 8 * T], sc[:, 6 * T : 7 * T], sc[:, 7 * T : 8 * T]
            )
            nc.vector.tensor_copy(mu[:], sc[:, 6 * T : 7 * T])
            bb = bcast(sc[:, 7 * T : 8 * T], "bc")
            for k in range(4):
                s = slice(k * T, (k + 1) * T)
                t3 = work.tile([128, T], F32, tag="t1")
                nc.vector.tensor_mul(t3[:], bb[:], ps[:, s])
                nc.vector.tensor_add(ps[:, s], rs[:, s], t3[:])
            allgather_p()

        # ---------------- epilogue: out = y - esc * x, AllGather full ----
        os_ = big.tile([128, 4 * T], F32)
        osb = big.tile([128, 4 * T], BF16)
        for k in range(4):
            s = slice(k * T, (k + 1) * T)
            u = work.tile([128, T], F32, tag="t1")
            nc.vector.tensor_scalar(u[:], xs[:, s], esc[:, k : k + 1], None, MULT)
            nc.vector.tensor_sub(os_[:, s], yc(k), u[:])
        nc.vector.tensor_copy(osb[:], os_[:])
        ago_in = dp.tile([R, T], BF16, name="ago_in")
        ago_out = dp.tile([N, T], BF16, addr_space="Shared", name="ago_out")
        nc.sync.dma_start(
            ago_in[:].rearrange("(k p) t -> p k t", p=128),
            osb[:].rearrange("p (k t) -> p k t", t=T),
        )
        nc.gpsimd.collective_compute(
            "AllGather",
            BYPASS,
            replica_groups=RG,
            ins=[ago_in.opt()],
            outs=[ago_out.opt()],
        )
        nc.sync.dma_start(out_d[:], ago_out[:])


class _Runner:
    """Caches the Bass module and the jitted PJRT executable so warm calls
    pay neither retrace nor NEFF recompile, and donates the previous call's
    device-resident output as the (fully overwritten) output scratch."""

    def __init__(self, niter):
        import jax
        from jax.sharding import Mesh, PartitionSpec
        from jax.experimental.shard_map import shard_map
        from concourse.bass2jax import (
            install_neuronx_cc_hook,
            _bass_exec_p,
            partition_id_tensor,
            fast_dispatch_compile,
        )
        import ml_dtypes

        self.jax = jax
        nc = _build(niter)
        self.nc = nc
        install_neuronx_cc_hook()

        partition_name = (
            nc.partition_id_tensor.name if nc.partition_id_tensor else None
        )
        in_names, out_names, out_avals = [], [], []
        zero_outs = []
        for alloc in nc.m.functions[0].allocations:
            if not isinstance(alloc, mybir.MemoryLocationSet):
                continue
            name = alloc.memorylocations[0].name
            if alloc.kind == "ExternalInput":
                if name != partition_name:
                    in_names.append(name)
            elif alloc.kind == "ExternalOutput":
                out_names.append(name)
                shape = tuple(alloc.tensor_shape)
                dtype = mybir.dt.np(alloc.dtype)
                out_avals.append(jax.core.ShapedArray(shape, dtype))
                zero_outs.append(np.zeros(shape, dtype))
        assert nc.dbg_addr is None
        n_params, n_outs = len(in_names), len(out_avals)
        in_names_full = list(in_names) + out_names
        if partition_name is not None:
            in_names_full.append(partition_name)
        donate = tuple(range(n_params, n_params + n_outs))
        self.in_names = in_names
        self.out_avals = out_avals
        self.zero_outs = zero_outs
        self.prev_out = None

        def _bodyf(*args):
            operands = list(args)
            if partition_name is not None:
                operands.append(partition_id_tensor())
            return tuple(
                _bass_exec_p.bind(
                    *operands,
                    out_avals=tuple(out_avals),
                    in_names=tuple(in_names_full),
                    out_names=tuple(out_names),
                    lowering_input_output_aliases=(),
                    sim_require_finite=True,
                    sim_require_nnan=True,
                    nc=nc,
                )
            )

        devices = jax.devices()[:C]
        assert len(devices) == C
        mesh = Mesh(np.asarray(devices), ("core",))
        in_specs = (PartitionSpec("core"),) * (n_params + n_outs)
        out_specs = (PartitionSpec("core"),) * n_outs
        from jax.sharding import NamedSharding

        self.sharding = NamedSharding(mesh, PartitionSpec("core"))
        sharded_jit = jax.jit(
            shard_map(
                _bodyf,
                mesh=mesh,
                in_specs=in_specs,
                out_specs=out_specs,
                check_rep=False,
            ),
            donate_argnums=donate,
            keep_unused=True,
        )
        # AOT-compile with bass_effect suppressed so calls take jax's C++
        # fast-path dispatch instead of the effectful Python path.
        arg_specs = [
            jax.ShapeDtypeStruct((N, D), ml_dtypes.bfloat16, sharding=self.sharding),
            jax.ShapeDtypeStruct((N, T), ml_dtypes.bfloat16, sharding=self.sharding),
            jax.ShapeDtypeStruct(
                (C * N, T), ml_dtypes.bfloat16, sharding=self.sharding
            ),
        ]
        self.sharded = fast_dispatch_compile(
            lambda: sharded_jit.lower(*arg_specs).compile()
        )

    def __call__(self, xb, yv):
        try:
            return self._run(xb, yv)
        except Exception:
            # A failed attempt may have consumed the donated scratch buffer;
            # drop it so the retry regenerates a fresh one.
            self.prev_out = None
            return self._run(xb, yv)

    def _run(self, xb, yv):
        scratch = self.prev_out
        if scratch is None or scratch.is_deleted():
            # device_put so the scratch arg is a sharded jax.Array on the
            # first call too — keeps the jit arg signature identical across
            # calls (no warm-call retrace when we start donating outputs).
            scratch = self.jax.device_put(
                np.zeros(
                    (C * self.out_avals[0].shape[0], *self.out_avals[0].shape[1:]),
                    self.out_avals[0].dtype,
                ),
                self.sharding,
            )
        out_arrs = self.sharded(xb, yv, scratch)
        self.prev_out = out_arrs[0]
        # every core holds the full AllGather'd output; fetch one shard only
        return np.asarray(out_arrs[0].addressable_shards[0].data)


class _Result:
    exec_time_ns = None
    profile_json = None


def kernel(X: np.ndarray, y: np.ndarray, niter: int = NITER, trace: bool = False):
    import ml_dtypes

    assert X.shape == (N, D) and y.shape == (N, T)
    xb = np.asarray(X).astype(ml_dtypes.bfloat16)  # row-sharded across cores
    yv = np.asarray(y).astype(ml_dtypes.bfloat16)

    if niter not in _CACHE:
        _CACHE[niter] = _Runner(niter)
        # Two extra executions on the build call: the first donates the
        # zero scratch, the second runs with the exact argument signature
        # (host arrays + device-resident donated output) that later calls
        # use, warming jax's C++ fast-path dispatch cache.
        _CACHE[niter](xb, yv)
        _CACHE[niter](xb, yv)
    runner = _CACHE[niter]

    out = runner(xb, yv).astype(np.float32)
    kernel.last_result = _Result()
    return out
